# revision 25
# baseline (speedup 1.0000x reference)
"""Trainium2 Bass kernel for nn_ConvMatrix2d (CapsNet-style matrix-capsule conv, k=1, s=1).

Computation (per batch b, input-capsule c, spatial position ji = J*14+I):
    out[b, c, o*196 + ji, p*4+r] = sum_q W[c,o,p,q] * x[b,c,I,J,q*4+r]
    out[b, c, o*196 + ji, 16]    = x[b,c,I,J,16]
Output: (32, 32, 6272, 17); HW computes fp16 x fp16 -> fp32 PSUM and stores
fp16 (rel err ~4e-4 vs the 2e-2 gate), host upcasts to fp32. This halves the
output-DMA bytes (437 MB -> 218 MB), which is the roofline resource.

Strategy (8 cores, data parallel over batch: 4 batches/core):
  - Host packs x into per-batch fp16 moving operands [16, 8*784]: rows
    (q, c_lo) hold pose[(ji, r)] per c_hi. Weights become block-diagonal
    fp16 stationaries w2[(c_hi,p)][16, 128] with w2[(q,cl), (cl',o)] =
    delta(cl,cl') * W[c,o,p,q], so ONE matmul of K=16, M=128, N=392
    computes a whole (p, half) for all 4 c_lo at once.
  - Acts are host-replicated across the 32 'o' partitions and copied
    SBUF->SBUF into the stage's t=16 column (no PSUM round-trip).
  - PSUM pair-tiles [128, 1024] (2 banks: p_even @ 0:392, p_odd @ 512:904),
    2 tiles per half, each drained by ONE interleaving copy
    (jj, p, r) -> contiguous (jj, t-octet) into the fp16 stage, DVE/ACT
    alternating.
  - 8-deep stage rotation; one 852KB out-DMA per (b, c_hi):
    128 partitions x 6664B contiguous descriptors.
"""

import numpy as np

import concourse.bass as bass
import concourse.bacc as bacc
import concourse.mybir as mybir
from concourse.tile import TileContext
from concourse.bass_utils import run_bass_kernel_spmd

# Problem constants (hardcoded per contract)
B, C, WSP, HH = 32, 32, 14, 17
O, H = 32, 4
JI = WSP * WSP          # 196
NB = 4                  # batches per core
NCORES = 8
CHI, CLO = 8, 4         # c = c_hi*4 + c_lo
HJ = 98                 # ji per half
FH = HJ * 4             # 392 moving cols per (p, half)
ROW = HH                # 17 output values per (c,o,ji)
SLAB = JI * HH          # 3332 per (b,c,o)

F32 = mybir.dt.float32
F16 = mybir.dt.float16


def _build_nc():
    nc = bacc.Bacc()
    x_d = nc.dram_tensor("x2", [NB, 16, CHI * 784], F16, kind="ExternalInput")
    xa_d = nc.dram_tensor("xa", [NB, 128, CHI * JI], F16, kind="ExternalInput")
    w_d = nc.dram_tensor("w2", [32, CHI * 2 * 128], F16, kind="ExternalInput")
    out_d = nc.dram_tensor("out", [NB, C, O * JI, HH], F16, kind="ExternalOutput")

    with TileContext(nc) as tc:
        with (
            tc.tile_pool(name="wpool", bufs=1) as wpool,
            tc.tile_pool(name="xpool", bufs=2) as xpool,
            tc.tile_pool(name="xapool", bufs=2) as xapool,
            tc.tile_pool(name="stage", bufs=8) as spool,
            tc.tile_pool(name="pp0", bufs=2, space="PSUM") as pp0pool,
            tc.tile_pool(name="pp1", bufs=2, space="PSUM") as pp1pool,
        ):
            w_sb = wpool.tile([80, CHI * 2 * 128], F16)
            nc.sync.dma_start(out=w_sb[0:16, :], in_=w_d[0:16, :])
            nc.sync.dma_start(out=w_sb[64:80, :], in_=w_d[16:32, :])

            x_sbs, xa_sbs = {}, {}

            def load_b(b):
                x_sbs[b] = xpool.tile([80, CHI * 784], F16, tag="x", name="x_sb")
                xa_sbs[b] = xapool.tile([128, CHI * JI], F16, tag="xa",
                                        name="xa_sb")
                # same x rows into PE row-strip 0 (p even) and strip 2 (p odd)
                nc.sync.dma_start(out=x_sbs[b][0:16, :], in_=x_d[b])
                nc.sync.dma_start(out=x_sbs[b][64:80, :], in_=x_d[b])
                nc.sync.dma_start(out=xa_sbs[b], in_=xa_d[b])

            load_b(0)
            ci = 0  # vote-copy engine alternation
            for b in range(NB):
                x_sb, xa_sb = x_sbs[b], xa_sbs[b]
                for chi in range(CHI):
                    stage = spool.tile([128, SLAB], F16, tag="stage")
                    st3 = stage.rearrange("z (ji t) -> z ji t", t=ROW)

                    # acts: SBUF->SBUF, no PSUM dependency; issue first
                    asrc = xa_sb[:, chi * JI:(chi + 1) * JI]
                    if chi % 2 == 0:
                        nc.vector.tensor_copy(st3[:, :, 16], asrc)
                    else:
                        nc.scalar.copy(st3[:, :, 16], asrc)

                    for h in range(2):
                        tiles = []
                        for pp in range(2):
                            pv = (pp0pool if pp == 0 else pp1pool).tile(
                                [128, 1024], F32, tag=f"pp{pp}")
                            tiles.append(pv)
                            # p = 2*pp + sub; even p on strip 0, odd on strip 2
                            # (different row groups -> HW-concurrent matmuls)
                            for sub in range(2):
                                base = 64 * sub
                                nc.tensor.matmul(
                                    pv[:, sub * 512:sub * 512 + FH],
                                    w_sb[base:base + 16,
                                         (chi * 2 + pp) * 128:
                                         (chi * 2 + pp + 1) * 128],
                                    x_sb[base:base + 16,
                                         chi * 784 + h * FH:
                                         chi * 784 + (h + 1) * FH],
                                )
                        # interleave PSUM -> stage (ji*17 + p*4 + r), f16
                        # src [128][jj 98 step 4][p 2 step 512][r 4 step 1]
                        # dst [128][jj 98 step 17][t 8 step 1] (same walk order)
                        for pp in range(2):
                            src = tiles[pp].rearrange(
                                "z (p jj r) -> z jj p r", p=2, r=4)[
                                :, 0:HJ, :, :]
                            dst = st3[:, h * HJ:(h + 1) * HJ,
                                      pp * 8:pp * 8 + 8]
                            if ci % 2 == 0:
                                nc.vector.tensor_copy(dst, src)
                            else:
                                nc.scalar.copy(dst, src)
                            ci += 1

                    if chi == 0 and b + 1 < NB:
                        load_b(b + 1)  # prefetch next batch early

                    # One 852KB out-DMA: dst [c_lo 4][o 32][3332 contig]
                    dst = out_d.rearrange(
                        "b (ch cl) (o j) t -> b ch cl o (j t)", cl=CLO, o=O
                    )[b, chi]
                    # alternate the two HWDGE rings (SP / ACT) so descriptor
                    # generation for consecutive out-DMAs overlaps
                    if chi % 2 == 0:
                        nc.sync.dma_start(out=dst, in_=stage[:])
                    else:
                        nc.scalar.dma_start(out=dst, in_=stage[:])
    if not nc.is_finalized():
        nc.finalize()
    return nc


_CACHE = {}


def _get_nc():
    if "nc" not in _CACHE:
        _CACHE["nc"] = _build_nc()
    return _CACHE["nc"]


def _preprocess(x, weight):
    """Build per-core input maps from full inputs."""
    x = np.ascontiguousarray(x, dtype=np.float32)
    weight = np.ascontiguousarray(weight, dtype=np.float32)
    xp = x.transpose(0, 1, 3, 2, 4).reshape(B, C, JI, HH)  # ji = J*14+I
    pose = xp[..., :16].reshape(B, CHI, 4, JI, 4, 4)       # [b,chi,cl,ji,q,r]
    # rows (q, c_lo) per c_hi, batched per b: [b, 16, (chi, ji, r)]
    x2 = np.ascontiguousarray(
        pose.transpose(0, 4, 2, 1, 3, 5).reshape(B, 16, CHI * 784)
    ).astype(np.float16)
    # acts replicated across the 32 'o' partitions: [b, (cl,o), (chi, ji)]
    xa = np.ascontiguousarray(np.broadcast_to(
        xp[..., 16].astype(np.float16).reshape(B, CHI, 4, 1, JI),
        (B, CHI, 4, O, JI)).transpose(0, 2, 3, 1, 4).reshape(B, 128, CHI * JI))

    Wm = weight[:, 0, 0]                                   # (C, O, 4, 4)
    A = Wm.reshape(CHI, 4, O, 4, 4).transpose(0, 3, 4, 1, 2)  # [chi,p,q,cl,o]
    w3 = np.zeros((CHI, 4, 4, 4, 4, O), dtype=np.float16)  # [chi,p,q,cl,cl',o]
    for cl in range(4):
        w3[:, :, :, cl, cl, :] = A[:, :, :, cl, :]
    # device layout: strip rows [0:16] hold p in {0,2} at free (chi, pp, 128),
    # rows [16:32] hold p in {1,3}; pp = p // 2 indexes within the strip.
    w4 = w3.reshape(CHI, 4, 16, 128)
    w2 = np.zeros((32, CHI * 2 * 128), dtype=np.float16)
    wv = w2.reshape(2, 16, CHI, 2, 128)
    for p in range(4):
        wv[p % 2, :, :, p // 2, :] = w4[:, p].transpose(1, 0, 2)
    w2 = np.ascontiguousarray(w2)

    in_maps = []
    for k in range(NCORES):
        in_maps.append({
            "x2": np.ascontiguousarray(x2[k * NB:(k + 1) * NB]),
            "xa": np.ascontiguousarray(xa[k * NB:(k + 1) * NB]),
            "w2": w2,
        })
    return in_maps


def _run(x, weight, trace=False, trace_kwargs=None):
    nc = _get_nc()
    in_maps = _preprocess(x, weight)
    res = run_bass_kernel_spmd(
        nc, in_maps, list(range(NCORES)), trace=trace,
        trace_kwargs=trace_kwargs or {},
    )
    out = np.concatenate([r["out"] for r in res.results], axis=0)
    return out.astype(np.float32), res


def kernel(x, weight):
    out, _ = _run(x, weight)
    return out


# revision 26
# speedup vs baseline: 1.1205x; 1.1205x over previous
"""Trainium2 Bass kernel for nn_ConvMatrix2d (CapsNet-style matrix-capsule conv, k=1, s=1).

Computation (per batch b, input-capsule c, spatial position ji = J*14+I):
    out[b, c, o*196 + ji, p*4+r] = sum_q W[c,o,p,q] * x[b,c,I,J,q*4+r]
    out[b, c, o*196 + ji, 16]    = x[b,c,I,J,16]
Output: (32, 32, 6272, 17); HW computes fp16 x fp16 -> fp32 PSUM and stores
fp16 (rel err ~4e-4 vs the 2e-2 gate), host upcasts to fp32. This halves the
output-DMA bytes (437 MB -> 218 MB), which is the roofline resource.

Strategy (8 cores, data parallel over batch: 4 batches/core):
  - Host packs x into per-batch fp16 moving operands [16, 8*784]: rows
    (q, c_lo) hold pose[(ji, r)] per c_hi. Weights become block-diagonal
    fp16 stationaries w2[(c_hi,p)][16, 128] with w2[(q,cl), (cl',o)] =
    delta(cl,cl') * W[c,o,p,q], so ONE matmul of K=16, M=128, N=392
    computes a whole (p, half) for all 4 c_lo at once.
  - Acts are host-replicated across the 32 'o' partitions and copied
    SBUF->SBUF into the stage's t=16 column (no PSUM round-trip).
  - PSUM pair-tiles [128, 1024] (2 banks: p_even @ 0:392, p_odd @ 512:904),
    2 tiles per half, each drained by ONE interleaving copy
    (jj, p, r) -> contiguous (jj, t-octet) into the fp16 stage, DVE/ACT
    alternating.
  - 8-deep stage rotation; one 852KB out-DMA per (b, c_hi):
    128 partitions x 6664B contiguous descriptors.
"""

import numpy as np

import concourse.bass as bass
import concourse.bacc as bacc
import concourse.mybir as mybir
from concourse.tile import TileContext
from concourse.bass_utils import run_bass_kernel_spmd

# Problem constants (hardcoded per contract)
B, C, WSP, HH = 32, 32, 14, 17
O, H = 32, 4
JI = WSP * WSP          # 196
NB = 4                  # batches per core
NCORES = 8
CHI, CLO = 8, 4         # c = c_hi*4 + c_lo
HJ = 98                 # ji per half
FH = HJ * 4             # 392 moving cols per (p, half)
ROW = HH                # 17 output values per (c,o,ji)
SLAB = JI * HH          # 3332 per (b,c,o)

F32 = mybir.dt.float32
F16 = mybir.dt.float16


def _build_nc():
    nc = bacc.Bacc()
    x_d = nc.dram_tensor("x2", [NB, 16, CHI * 784], F16, kind="ExternalInput")
    xa_d = nc.dram_tensor("xa", [NB, 128, CHI * JI], F16, kind="ExternalInput")
    w_d = nc.dram_tensor("w2", [32, CHI * 2 * 128], F16, kind="ExternalInput")
    out_d = nc.dram_tensor("out", [NB, C, O * JI, HH], F16, kind="ExternalOutput")

    with TileContext(nc) as tc:
        with (
            tc.tile_pool(name="wpool", bufs=1) as wpool,
            tc.tile_pool(name="xpool", bufs=2) as xpool,
            tc.tile_pool(name="xapool", bufs=2) as xapool,
            tc.tile_pool(name="stage", bufs=8) as spool,
            tc.tile_pool(name="pp0", bufs=2, space="PSUM") as pp0pool,
            tc.tile_pool(name="pp1", bufs=2, space="PSUM") as pp1pool,
        ):
            w_sb = wpool.tile([80, CHI * 2 * 128], F16)
            nc.sync.dma_start(out=w_sb[0:16, :], in_=w_d[0:16, :])
            nc.sync.dma_start(out=w_sb[64:80, :], in_=w_d[16:32, :])

            x_sbs, xa_sbs = {}, {}

            def load_b(b):
                x_sbs[b] = xpool.tile([80, CHI * 784], F16, tag="x", name="x_sb")
                xa_sbs[b] = xapool.tile([128, CHI * JI], F16, tag="xa",
                                        name="xa_sb")
                # same x rows into PE row-strip 0 (p even) and strip 2 (p odd)
                nc.sync.dma_start(out=x_sbs[b][0:16, :], in_=x_d[b])
                nc.sync.dma_start(out=x_sbs[b][64:80, :], in_=x_d[b])
                nc.sync.dma_start(out=xa_sbs[b], in_=xa_d[b])

            load_b(0)
            ci = 0  # vote-copy engine alternation
            for b in range(NB):
                x_sb, xa_sb = x_sbs[b], xa_sbs[b]
                for chi in range(CHI):
                    stage = spool.tile([128, SLAB], F16, tag="stage")
                    st3 = stage.rearrange("z (ji t) -> z ji t", t=ROW)

                    # acts: SBUF->SBUF, no PSUM dependency; issue first
                    asrc = xa_sb[:, chi * JI:(chi + 1) * JI]
                    if chi % 2 == 0:
                        nc.vector.tensor_copy(st3[:, :, 16], asrc)
                    else:
                        nc.scalar.copy(st3[:, :, 16], asrc)

                    for h in range(2):
                        tiles = []
                        for pp in range(2):
                            pv = (pp0pool if pp == 0 else pp1pool).tile(
                                [128, 1024], F32, tag=f"pp{pp}")
                            tiles.append(pv)
                            # p = 2*pp + sub; even p on strip 0, odd on strip 2
                            # (different row groups -> HW-concurrent matmuls)
                            for sub in range(2):
                                base = 64 * sub
                                nc.tensor.matmul(
                                    pv[:, sub * 512:sub * 512 + FH],
                                    w_sb[base:base + 16,
                                         (chi * 2 + pp) * 128:
                                         (chi * 2 + pp + 1) * 128],
                                    x_sb[base:base + 16,
                                         chi * 784 + h * FH:
                                         chi * 784 + (h + 1) * FH],
                                )
                        # interleave PSUM -> stage (ji*17 + p*4 + r), f16
                        # src [128][jj 98 step 4][p 2 step 512][r 4 step 1]
                        # dst [128][jj 98 step 17][t 8 step 1] (same walk order)
                        for pp in range(2):
                            src = tiles[pp].rearrange(
                                "z (p jj r) -> z jj p r", p=2, r=4)[
                                :, 0:HJ, :, :]
                            dst = st3[:, h * HJ:(h + 1) * HJ,
                                      pp * 8:pp * 8 + 8]
                            if ci % 2 == 0:
                                nc.vector.tensor_copy(dst, src)
                            else:
                                nc.scalar.copy(dst, src)
                            ci += 1

                    if chi == 0 and b + 1 < NB:
                        load_b(b + 1)  # prefetch next batch early

                    # One 852KB out-DMA: dst [c_lo 4][o 32][3332 contig]
                    dst = out_d.rearrange(
                        "b (ch cl) (o j) t -> b ch cl o (j t)", cl=CLO, o=O
                    )[b, chi]
                    nc.sync.dma_start(out=dst, in_=stage[:])
    if not nc.is_finalized():
        nc.finalize()
    return nc


_CACHE = {}


def _get_nc():
    if "nc" not in _CACHE:
        _CACHE["nc"] = _build_nc()
    return _CACHE["nc"]


def _preprocess(x, weight):
    """Build per-core input maps from full inputs."""
    x = np.ascontiguousarray(x, dtype=np.float32)
    weight = np.ascontiguousarray(weight, dtype=np.float32)
    xp = x.transpose(0, 1, 3, 2, 4).reshape(B, C, JI, HH)  # ji = J*14+I
    pose = xp[..., :16].reshape(B, CHI, 4, JI, 4, 4)       # [b,chi,cl,ji,q,r]
    # rows (q, c_lo) per c_hi, batched per b: [b, 16, (chi, ji, r)]
    x2 = np.ascontiguousarray(
        pose.transpose(0, 4, 2, 1, 3, 5).reshape(B, 16, CHI * 784)
    ).astype(np.float16)
    # acts replicated across the 32 'o' partitions: [b, (cl,o), (chi, ji)]
    xa = np.ascontiguousarray(np.broadcast_to(
        xp[..., 16].astype(np.float16).reshape(B, CHI, 4, 1, JI),
        (B, CHI, 4, O, JI)).transpose(0, 2, 3, 1, 4).reshape(B, 128, CHI * JI))

    Wm = weight[:, 0, 0]                                   # (C, O, 4, 4)
    A = Wm.reshape(CHI, 4, O, 4, 4).transpose(0, 3, 4, 1, 2)  # [chi,p,q,cl,o]
    w3 = np.zeros((CHI, 4, 4, 4, 4, O), dtype=np.float16)  # [chi,p,q,cl,cl',o]
    for cl in range(4):
        w3[:, :, :, cl, cl, :] = A[:, :, :, cl, :]
    # device layout: strip rows [0:16] hold p in {0,2} at free (chi, pp, 128),
    # rows [16:32] hold p in {1,3}; pp = p // 2 indexes within the strip.
    w4 = w3.reshape(CHI, 4, 16, 128)
    w2 = np.zeros((32, CHI * 2 * 128), dtype=np.float16)
    wv = w2.reshape(2, 16, CHI, 2, 128)
    for p in range(4):
        wv[p % 2, :, :, p // 2, :] = w4[:, p].transpose(1, 0, 2)
    w2 = np.ascontiguousarray(w2)

    in_maps = []
    for k in range(NCORES):
        in_maps.append({
            "x2": np.ascontiguousarray(x2[k * NB:(k + 1) * NB]),
            "xa": np.ascontiguousarray(xa[k * NB:(k + 1) * NB]),
            "w2": w2,
        })
    return in_maps


def _run(x, weight, trace=False, trace_kwargs=None):
    nc = _get_nc()
    in_maps = _preprocess(x, weight)
    res = run_bass_kernel_spmd(
        nc, in_maps, list(range(NCORES)), trace=trace,
        trace_kwargs=trace_kwargs or {},
    )
    out = np.concatenate([r["out"] for r in res.results], axis=0)
    return out.astype(np.float32), res


def kernel(x, weight):
    out, _ = _run(x, weight)
    return out
